# revision 1
# baseline (speedup 1.0000x reference)
"""Trainium2 Bass kernel for nn_CNNEncoder (gather -> lin1 -> conv1d -> maxpool -> MLP).

Strategy
--------
Data-parallel over the 1024 = 64*16 sentences: 128 sentences per NeuronCore.

Host-side algebra: the conv (VALID, taps k=0..4) consumes lin1's output, so
lin1 can be folded into the conv weights:
    y[n,:,t] = sum_k (e[n,t+k] @ W1 + b1) @ Wk + conv_b
             = sum_k e[n,t+k] @ (W1 @ Wk)  +  (b1 @ sum_k Wk + conv_b)
with Wk[i,o] = conv_w[o,i,k].  The constant bias `beff` commutes with the
max-over-time, so it is folded into the MLP bias: b2eff = b2 + beff @ W2[:D].

Per core:
  1. indirect-DMA gather of embedding rows (padded to 320 f32 so each row is a
     256B-multiple) -> SBUF [pos=128, sent, 320]
  2. PE transposes -> e_T [ch_chunk, sent, pos]  (channel-major)
  3. conv as 15 PSUM-accumulated matmuls per (o_chunk, 4-sentence block):
     lhsT = Weff_k[i_chunk, o_chunk], rhs = shifted window of e_T, N = 4*124
  4. DVE max over time -> cnn_T [o_chunk, sent]
  5. tail MLP entirely in [ch, sent] layout (tanh via ACT with per-partition
     bias); output written transposed, un-transposed on host.
"""

import sys

sys.path.insert(0, "/opt/trn_rl_repo")

import os
from contextlib import ExitStack

import numpy as np

import concourse.bass as bass
import concourse.mybir as mybir
import concourse.tile as tile
from concourse import bacc, bass_utils

F32 = mybir.dt.float32
F32R = mybir.dt.float32r
BF16 = mybir.dt.bfloat16
I32 = mybir.dt.int32

VOCAB = 100000
D = 300
K = 5
L = 128          # tokens per sentence
NSENT = 1024     # total sentences
NCORES = 8
NS = NSENT // NCORES   # sentences per core = 128
SB = 4                 # sentences per block
NB = NS // SB          # 32 blocks
TP = L - K + 1         # 124 valid conv positions
DPAD = 320             # embedding row padded to 320 f32 = 1280B (256B multiple)
CH = [(0, 128), (128, 256), (256, 300)]  # chunking of the 300-dim channel axes

# 'f32' (exact, 4 cyc/row), 'f32r' (full rate at N>=256), 'bf16'
CONV_DTYPE = os.environ.get("BASS_CONV_DTYPE", "f32r")

_PROGRAM_CACHE = {}


def _build_program(conv_dtype: str) -> bass.Bass:
    nc = bacc.Bacc(None, target_bir_lowering=False)

    w_dt = {"bf16": BF16, "f32r": F32R, "f32": F32}[conv_dtype]
    # dtype of the gather->transpose path; f32r makes PE transposes 1.5 vs 2
    # cycles/row (PE rounds on read, so no extra precision loss vs f32r matmul)
    tr_dt = F32R if conv_dtype == "f32r" else F32

    # ---- per-core DRAM I/O ----
    tid_t = nc.dram_tensor("tid", [L, NS], I32, kind="ExternalInput")       # [pos, sent]
    embp = nc.dram_tensor("embp", [VOCAB, DPAD], tr_dt, kind="ExternalInput")
    weff = nc.dram_tensor("weff", [K, D, D], w_dt, kind="ExternalInput")    # [k, i, o]
    # packed residual conv weights for channels 256:300 (see conv loop):
    # wr01 rows {0:44 -> tap0, 64:108 -> tap1}, wr23 likewise taps 2/3,
    # wr4 rows 0:44 -> tap 4.  Zero rows elsewhere.
    wr01 = nc.dram_tensor("wr01", [128, D], w_dt, kind="ExternalInput")
    wr23 = nc.dram_tensor("wr23", [128, D], w_dt, kind="ExternalInput")
    wr4 = nc.dram_tensor("wr4", [64, D], w_dt, kind="ExternalInput")
    idn = nc.dram_tensor("idn", [L, L], tr_dt, kind="ExternalInput")        # identity
    idsh = nc.dram_tensor("idsh", [L, L], tr_dt, kind="ExternalInput")      # shift-1 identity
    # tail weights with biases folded in as an extra contraction row:
    # w2cat = [W2 (600 rows); b2eff] -> [601, D], w3cat = [W3; b3] -> [301, D]
    w2cat = nc.dram_tensor("w2cat", [2 * D + 1, D], F32R, kind="ExternalInput")
    w3cat = nc.dram_tensor("w3cat", [D + 1, D], F32R, kind="ExternalInput")
    # mention_rep transposed, with a trailing all-ones row (drives the bias rows)
    m_t = nc.dram_tensor("mt", [D + 1, NS], F32R, kind="ExternalInput")     # [ch, sent]
    out_d = nc.dram_tensor("out", [NS, D], F32, kind="ExternalOutput")      # [sent, ch]

    with tile.TileContext(nc) as tc, ExitStack() as ctx:
        const = ctx.enter_context(tc.tile_pool(name="const", bufs=1))
        epool = ctx.enter_context(tc.tile_pool(name="e", bufs=12))
        etpool = ctx.enter_context(tc.tile_pool(name="et", bufs=6))
        pspool = ctx.enter_context(tc.tile_pool(name="ps", bufs=8, space="PSUM"))

        ident = const.tile([128, 128], tr_dt)
        nc.sync.dma_start(out=ident[:], in_=idn[:])
        ident_s1 = const.tile([L, L], tr_dt)
        nc.sync.dma_start(out=ident_s1[:], in_=idsh[:])

        tid_sb = const.tile([L, NS], I32)
        nc.sync.dma_start(out=tid_sb[:], in_=tid_t[:])

        weff_sb = []  # [k][ci] -> [128, D] for the two full 128-channel chunks
        for k in range(K):
            per_c = []
            for c0, c1 in CH[:2]:
                t = const.tile([c1 - c0, D], w_dt, tag=f"weff{k}_{c0}")
                nc.sync.dma_start(out=t[:], in_=weff[k, c0:c1, :])
                per_c.append(t)
            weff_sb.append(per_c)
        wr01_sb = const.tile([128, D], w_dt)
        nc.sync.dma_start(out=wr01_sb[:], in_=wr01[:])
        wr23_sb = const.tile([128, D], w_dt)
        nc.sync.dma_start(out=wr23_sb[:], in_=wr23[:])
        wr4_sb = const.tile([64, D], w_dt)
        nc.sync.dma_start(out=wr4_sb[:], in_=wr4[:])

        # concat_T tiles [i-chunk, sent] for the tail contraction over the
        # 601-row [cnn(300); mention(300); ones] stack.  cnn rows are written
        # by the conv reduce_max; mention/ones rows DMA'd from m_t.
        W2CH = [(0, 128), (128, 256), (256, 384), (384, 512), (512, 601)]
        c_sb = [
            const.tile([c1 - c0, NS], F32R, tag=f"c_{c0}", name=f"c_{c0}")
            for c0, c1 in W2CH
        ]
        nc.sync.dma_start(out=c_sb[2][44:128, :], in_=m_t[0:84, :])
        nc.sync.dma_start(out=c_sb[3][:], in_=m_t[84:212, :])
        nc.sync.dma_start(out=c_sb[4][:], in_=m_t[212:301, :])

        w2cat_sb = []
        for c0, c1 in W2CH:
            t = const.tile([c1 - c0, D], F32R, tag=f"w2c_{c0}", name=f"w2c_{c0}")
            nc.sync.dma_start(out=t[:], in_=w2cat[c0:c1, :])
            w2cat_sb.append(t)

        JCH = [(0, 100), (100, 200), (200, 300)]
        w3cat_sb = []
        for j0, j1 in JCH:
            t = const.tile([j1 - j0, D], F32R, tag=f"w3c_{j0}", name=f"w3c_{j0}")
            nc.sync.dma_start(out=t[:], in_=w3cat[j0:j1, :])
            w3cat_sb.append(t)
        b3row_sb = const.tile([1, D], F32R)
        nc.sync.dma_start(out=b3row_sb[:], in_=w3cat[D : D + 1, :])
        ones_sb = const.tile([1, NS], F32R)
        nc.sync.dma_start(out=ones_sb[:], in_=m_t[D : D + 1, :])

        # ---- main loop over 4-sentence blocks ----
        for b in range(NB):
            # one gather per sentence: idx [128, 1], out [128, DPAD]
            # (multi-index-per-partition indirect DMA is broken on HW)
            e_s = []
            for s in range(SB):
                e_t = epool.tile([L, DPAD], tr_dt, tag="e", name=f"e_{b}_{s}")
                col = b * SB + s
                nc.gpsimd.indirect_dma_start(
                    out=e_t[:],
                    out_offset=None,
                    in_=embp[:],
                    in_offset=bass.IndirectOffsetOnAxis(
                        ap=tid_sb[:, col : col + 1], axis=0
                    ),
                )
                e_s.append(e_t)

            # transpose to channel-major.
            # et[0], et[1]: channels 0:128 / 128:256, [128, SB, 128].
            # et2: rows 0:64 = channels 256:320 (300:320 are zero-padded),
            #      rows 64:128 = same channels POSITION-SHIFTED by +1 (via the
            #      shift-1 identity) so two conv taps can share one matmul.
            et = []
            for ci, (c0, c1) in enumerate(CH[:2]):
                ps_tr = pspool.tile([128, SB, L], tr_dt, tag="ps", name=f"ps_tr{ci}")
                for s in range(SB):
                    nc.tensor.transpose(
                        out=ps_tr[:, s, :],
                        in_=e_s[s][:, c0:c1],
                        identity=ident[:],
                    )
                et_c = etpool.tile([128, SB, L], w_dt, tag="et", name=f"et{ci}")
                nc.scalar.copy(out=et_c[:], in_=ps_tr[:])
                et.append(et_c)
            # band A (unshifted) and band B (pos+1, via shifted identity) both
            # transpose to PSUM base partition 0 (4-byte transposes may not
            # write at a partition offset); band B is then partition-shifted
            # into et2[64:128] by a SBUF->SBUF DMA.
            ps_tr2a = pspool.tile([64, SB, L], tr_dt, tag="ps")
            ps_tr2b = pspool.tile([64, SB, L], tr_dt, tag="ps")
            for s in range(SB):
                nc.tensor.transpose(
                    out=ps_tr2a[:, s, :], in_=e_s[s][:, 256:320], identity=ident[:]
                )
                nc.tensor.transpose(
                    out=ps_tr2b[:, s, :], in_=e_s[s][:, 256:320], identity=ident_s1[:]
                )
            et2 = etpool.tile([128, SB, L], w_dt, tag="et")
            etb = etpool.tile([64, SB, L], w_dt, tag="etb")
            nc.scalar.copy(out=et2[0:64], in_=ps_tr2a[:])
            nc.scalar.copy(out=etb[:], in_=ps_tr2b[:])
            nc.sync.dma_start(out=et2[64:128], in_=etb[:])

            # conv: 13 PSUM-accumulated matmuls per o_chunk, then max over time
            for oi, (o0, o1) in enumerate(CH):
                ps_y = pspool.tile([o1 - o0, SB, TP], F32, tag="ps")
                n = 0

                def mm(lhsT, rhs, idx):
                    nc.tensor.matmul(
                        out=ps_y[:], lhsT=lhsT, rhs=rhs,
                        start=(idx == 0), stop=(idx == 12),
                    )

                for ci in range(2):
                    for k in range(K):
                        mm(weff_sb[k][ci][:, o0:o1], et[ci][:, :, k : k + TP], n)
                        n += 1
                # channels 256:300, taps packed: (0,1), (2,3), (4)
                mm(wr01_sb[:, o0:o1], et2[:, :, 0:TP], n); n += 1
                mm(wr23_sb[:, o0:o1], et2[:, :, 2 : 2 + TP], n); n += 1
                mm(wr4_sb[:, o0:o1], et2[0:64, :, 4 : 4 + TP], n); n += 1
                cnn_rows = c_sb[oi][0 : o1 - o0] if oi == 2 else c_sb[oi][:]
                nc.vector.tensor_reduce(
                    out=cnn_rows[:, b * SB : (b + 1) * SB],
                    in_=ps_y[:],
                    axis=mybir.AxisListType.X,
                    op=mybir.AluOpType.max,
                )

        # ---- tail MLP, f32r full-rate (N=300), biases folded as ones-rows ----
        # h[s, j] = tanh(sum_c concat_T[c, s] * w2cat[c, j])
        ps_h = pspool.tile([NS, D], F32, tag="ps")
        for c, (c0, c1) in enumerate(W2CH):
            nc.tensor.matmul(
                out=ps_h[:],
                lhsT=c_sb[c][:],
                rhs=w2cat_sb[c][:],
                start=(c == 0),
                stop=(c == len(W2CH) - 1),
            )
        h_sb = const.tile([NS, D], F32R)
        nc.scalar.activation(
            out=h_sb[:], in_=ps_h[:], func=mybir.ActivationFunctionType.Tanh
        )

        # transpose h -> h_T [j-chunk, s] for the second contraction
        ht_sb = []
        for jc, (j0, j1) in enumerate(JCH):
            ps_ht = pspool.tile([100, NS], F32R, tag="ps")
            nc.tensor.transpose(out=ps_ht[:], in_=h_sb[:, j0:j1], identity=ident[:])
            ht = const.tile([100, NS], F32R, tag=f"ht_{j0}", name=f"ht_{j0}")
            nc.scalar.copy(out=ht[:], in_=ps_ht[:])
            ht_sb.append(ht)

        # out[s, q] = sum_j h_T[j, s] * w3cat[j, q] + ones[s] * b3[q]
        ps_o = pspool.tile([NS, D], F32, tag="ps")
        for jc in range(3):
            nc.tensor.matmul(
                out=ps_o[:],
                lhsT=ht_sb[jc][:],
                rhs=w3cat_sb[jc][:],
                start=(jc == 0),
                stop=False,
            )
        nc.tensor.matmul(
            out=ps_o[:], lhsT=ones_sb[:], rhs=b3row_sb[:], start=False, stop=True
        )
        out_sb = const.tile([NS, D], F32)
        nc.scalar.copy(out=out_sb[:], in_=ps_o[:])
        nc.sync.dma_start(out=out_d[:], in_=out_sb[:])

    nc.finalize()
    return nc


def get_program(conv_dtype: str = CONV_DTYPE) -> bass.Bass:
    if conv_dtype not in _PROGRAM_CACHE:
        _PROGRAM_CACHE[conv_dtype] = _build_program(conv_dtype)
    return _PROGRAM_CACHE[conv_dtype]


def _prepare_in_maps(inputs: dict) -> list[dict]:
    token_ids = np.asarray(inputs["token_ids"]).astype(np.int32)      # [1024, 128]
    mention = np.asarray(inputs["mention_rep"], dtype=np.float32).reshape(NSENT, D)
    emb = np.asarray(inputs["emb"], dtype=np.float32)
    W1 = np.asarray(inputs["W1"], dtype=np.float64)
    b1 = np.asarray(inputs["b1"], dtype=np.float64)
    conv_w = np.asarray(inputs["conv_w"], dtype=np.float64)           # [o, i, k]
    conv_b = np.asarray(inputs["conv_b"], dtype=np.float64)
    W2 = np.asarray(inputs["W2"], dtype=np.float64)                   # [2D, D]
    b2 = np.asarray(inputs["b2"], dtype=np.float64)
    W3 = np.asarray(inputs["W3"], dtype=np.float32)                   # [j, q]
    b3 = np.asarray(inputs["b3"], dtype=np.float32)

    Wk = conv_w.transpose(1, 0, 2)                                    # [i, o, k]
    weff = np.stack([W1 @ Wk[:, :, k] for k in range(K)])             # [k, i, o]
    beff = b1 @ Wk.sum(axis=2) + conv_b                               # [o]
    b2eff = b2 + beff @ W2[:D]                                        # [j]
    w2cat_h = np.concatenate([W2, b2eff[None, :]], axis=0).astype(np.float32)
    w3cat_h = np.concatenate(
        [W3.astype(np.float64), np.asarray(inputs["b3"], np.float64)[None, :]], axis=0
    ).astype(np.float32)

    wdt = np.float32
    if CONV_DTYPE == "bf16":
        import ml_dtypes

        wdt = ml_dtypes.bfloat16
    weff_h = weff.astype(wdt)

    # packed residual weights (channels 256:300) — layouts match the kernel's
    # et2 tile: rows 0:44 tap A, 64:108 tap B (shift +1 baked into et2 rows 64+)
    wr01_h = np.zeros((128, D), wdt)
    wr01_h[0:44] = weff[0, 256:300].astype(wdt)
    wr01_h[64:108] = weff[1, 256:300].astype(wdt)
    wr23_h = np.zeros((128, D), wdt)
    wr23_h[0:44] = weff[2, 256:300].astype(wdt)
    wr23_h[64:108] = weff[3, 256:300].astype(wdt)
    wr4_h = np.zeros((64, D), wdt)
    wr4_h[0:44] = weff[4, 256:300].astype(wdt)

    # cyclic shift-1 permutation: ones at ((c+1) % L, c) bakes out[:, c] = in[c+1]
    # (column L-1 wraps to position 0 but is never consumed by the conv windows)
    idsh_h = np.zeros((L, L), np.float32)
    idsh_h[(np.arange(L) + 1) % L, np.arange(L)] = 1.0
    idn_h = np.eye(L, dtype=np.float32)

    emb_pad = np.zeros((VOCAB, DPAD), dtype=np.float32)
    emb_pad[:, :D] = emb

    in_maps = []
    for c in range(NCORES):
        sl = slice(c * NS, (c + 1) * NS)
        mt_h = np.ones((D + 1, NS), np.float32)
        mt_h[:D] = mention[sl].T
        in_maps.append(
            {
                "tid": np.ascontiguousarray(token_ids[sl].T),
                "embp": emb_pad,
                "weff": weff_h,
                "wr01": wr01_h,
                "wr23": wr23_h,
                "wr4": wr4_h,
                "idn": idn_h,
                "idsh": idsh_h,
                "w2cat": w2cat_h,
                "w3cat": w3cat_h,
                "mt": mt_h,
            }
        )
    return in_maps


def run(inputs: dict, trace: bool = False, **kwargs):
    """Run the kernel; returns (output [1024, 300] f32, BassKernelResults)."""
    nc = get_program()
    in_maps = _prepare_in_maps(inputs)
    res = bass_utils.run_bass_kernel_spmd(
        nc, in_maps, core_ids=list(range(NCORES)), trace=trace, **kwargs
    )
    out = np.concatenate(
        [np.asarray(r["out"]) for r in res.results], axis=0
    ).astype(np.float32)
    return out, res


def kernel(**inputs) -> np.ndarray:
    out, _ = run(inputs)
    return out



# revision 10
# speedup vs baseline: 2.0940x; 2.0940x over previous
"""Trainium2 Bass kernel for nn_CNNEncoder (gather -> lin1 -> conv1d -> maxpool -> MLP).

Strategy (v2: fp8 DoubleRow + transposing dma_gather)
-----------------------------------------------------
Data-parallel over the 1024 = 64*16 sentences: 128 sentences per NeuronCore.

Host-side algebra: lin1 folds into the conv weights (weff[k] = W1 @ Wk), the
constant bias folds into the MLP bias (as in v1).

Precision: the conv runs on the PE in fp8-e4m3 with perf_mode=DoubleRow
(0.5 cycles/row, two 128-row k-tiles per stream).  A single fp8 pass is just
over the error budget, so each embedding row carries a RESIDUAL: the table
row is [fp8(e) | fp8(32*(e - fp8(e)))] and the conv contracts over both the
hi and lo halves (3000 rows -> 12 DoubleRow streams per o_chunk).  Weights:
hi rows use fp8(32*weff), lo rows fp8(weff); the PSUM then holds 32*y and the
1/32 is folded into W2's cnn rows.  Measured end-to-end rel-err ~ 0.019.

Gather: per-core the <=16384 distinct tokens are host-compacted into a
[16384, 768B] fp8 table (int16-indexable), and ONE transpose-mode dma_gather
per 2048-token chunk lands the data channel-major in SBUF (u16-granularity
transpose; channel pairs ride the fp8 parity axis, which matches DoubleRow's
two k-tiles).  This removes all PE transposes and the per-sentence SWDGE
overhead of v1 (8 x ~1.8us Pool instead of 128 x ~1.04us).

Conv: sentences are processed on a 512-token "long axis" (4 sentences/block,
N=508 positions incl. cross-sentence garbage); the max-reduce reads only the
124 valid positions per sentence via a strided AP.  Leftover channelxtap rows
(lo ch 212:299 x 5 taps) are partition-packed by 6 small SBUF->SBUF DMAs so
every o_chunk needs exactly ceil(3000/256) = 12 streams.
"""

import sys

sys.path.insert(0, "/opt/trn_rl_repo")

from contextlib import ExitStack

import ml_dtypes
import numpy as np

import concourse.bass as bass
import concourse.mybir as mybir
import concourse.tile as tile
from concourse import bacc, bass_utils

F32 = mybir.dt.float32
F32R = mybir.dt.float32r
F8 = mybir.dt.float8e4
I16 = mybir.dt.int16
E4 = ml_dtypes.float8_e4m3fn
DR = mybir.MatmulPerfMode.DoubleRow

VOCAB = 100000
D = 300
K = 5
L = 128            # tokens per sentence
NSENT = 1024
NCORES = 8
NS = NSENT // NCORES     # sentences per core = 128
NTOK = NS * L            # tokens per core = 16384
NR = NTOK                # compact table rows (padded)
ES = 768                 # table row bytes: hi 300 | lo 300 | pad 168
GCH = 32                 # gather chunks (one conv block each; NI>768 crashes Q7)
NI = NTOK // GCH         # idxs per gather = 512
BLK_TOK = 512            # tokens per conv block (4 sentences)
NBLK_G = NI // BLK_TOK   # blocks per chunk = 4
SB = BLK_TOK // L        # sentences per block = 4
TP = L - K + 1           # 124 valid conv positions per sentence
N = BLK_TOK - (K - 1)    # 508 conv positions per block stream
ETPAD = 16               # tail pad so shifted window reads stay in-tile
CH = [(0, 128), (128, 256), (256, 300)]
W2CH = [(0, 128), (128, 256), (256, 384), (384, 512), (512, 601)]
JCH = [(0, 100), (100, 200), (200, 300)]

_PROGRAM_CACHE = {}


def _build_program() -> bass.Bass:
    nc = bacc.Bacc(None, target_bir_lowering=False, dynamic_dma_scratch_size=32768)

    tbl = nc.dram_tensor("tbl", [NR, ES], F8, kind="ExternalInput")
    idx = nc.dram_tensor("idx", [128, NTOK // 16], I16, kind="ExternalInput")
    wf0 = nc.dram_tensor("wf0", [128, K, 3, 2, 128], F8, kind="ExternalInput")
    wf1 = nc.dram_tensor("wf1", [128, K, 3, 2, 128], F8, kind="ExternalInput")
    wa = nc.dram_tensor("wa", [128, 3, 2, 128], F8, kind="ExternalInput")
    wb = nc.dram_tensor("wb", [92, 3, 2, 128], F8, kind="ExternalInput")
    idn = nc.dram_tensor("idn", [128, 128], F32R, kind="ExternalInput")
    w2cat = nc.dram_tensor("w2cat", [2 * D + 1, D], F32R, kind="ExternalInput")
    w3cat = nc.dram_tensor("w3cat", [D + 1, D], F32R, kind="ExternalInput")
    m_t = nc.dram_tensor("mt", [D + 1, NS], F32R, kind="ExternalInput")
    out_d = nc.dram_tensor("out", [NS, D], F32, kind="ExternalOutput")

    with tile.TileContext(nc) as tc, ExitStack() as ctx:
        const = ctx.enter_context(tc.tile_pool(name="const", bufs=1))
        etpool = ctx.enter_context(tc.tile_pool(name="et", bufs=3))
        pkpool = ctx.enter_context(tc.tile_pool(name="pk", bufs=3))
        pspool = ctx.enter_context(tc.tile_pool(name="ps", bufs=8, space="PSUM"))

        idx_sb = const.tile([128, NTOK // 16], I16)
        nc.sync.dma_start(out=idx_sb[:], in_=idx[:])
        wf0_sb = const.tile([128, K, 3, 2, 128], F8)
        nc.sync.dma_start(out=wf0_sb[:], in_=wf0[:])
        wf1_sb = const.tile([128, K, 3, 2, 128], F8)
        nc.sync.dma_start(out=wf1_sb[:], in_=wf1[:])
        wa_sb = const.tile([128, 3, 2, 128], F8)
        nc.sync.dma_start(out=wa_sb[:], in_=wa[:])
        wb_sb = const.tile([92, 3, 2, 128], F8)
        nc.sync.dma_start(out=wb_sb[:], in_=wb[:])
        ident = const.tile([128, 128], F32R)
        nc.sync.dma_start(out=ident[:], in_=idn[:])

        # concat_T tiles [row-chunk, sent] for the tail contraction over the
        # 601-row [32*cnn(300); mention(300); ones] stack.
        c_sb = [
            const.tile([c1 - c0, NS], F32R, tag=f"c_{c0}", name=f"c_{c0}")
            for c0, c1 in W2CH
        ]
        nc.sync.dma_start(out=c_sb[2][44:128, :], in_=m_t[0:84, :])
        nc.sync.dma_start(out=c_sb[3][:], in_=m_t[84:212, :])
        nc.sync.dma_start(out=c_sb[4][:], in_=m_t[212:301, :])

        w2cat_sb = []
        for c0, c1 in W2CH:
            t = const.tile([c1 - c0, D], F32R, tag=f"w2c_{c0}", name=f"w2c_{c0}")
            nc.sync.dma_start(out=t[:], in_=w2cat[c0:c1, :])
            w2cat_sb.append(t)
        w3cat_sb = []
        for j0, j1 in JCH:
            t = const.tile([j1 - j0, D], F32R, tag=f"w3c_{j0}", name=f"w3c_{j0}")
            nc.sync.dma_start(out=t[:], in_=w3cat[j0:j1, :])
            w3cat_sb.append(t)
        b3row_sb = const.tile([1, D], F32R)
        nc.sync.dma_start(out=b3row_sb[:], in_=w3cat[D : D + 1, :])
        ones_sb = const.tile([1, NS], F32R)
        nc.sync.dma_start(out=ones_sb[:], in_=m_t[D : D + 1, :])

        def dr_rhs(tile_ap, base):
            # [128, 2, N] window: parity stride 1, token stride 2
            win = tile_ap[:, base : base + 2 * N]
            return win.rearrange("p (n two) -> p two n", two=2)

        dma_engines = [nc.sync, nc.scalar]
        nred = 0
        for g in range(GCH):
            et = etpool.tile([128, 3 * 2 * NI + ETPAD], F8, tag="et")
            nc.vector.memset(et[:, 3 * 2 * NI : 3 * 2 * NI + ETPAD], 0)
            gout = et[:, 0 : 3 * 2 * NI].rearrange("p (j i) -> p j i", j=6)
            nc.gpsimd.dma_gather(
                gout, tbl[:], idx_sb[:, g * (NI // 16) : (g + 1) * (NI // 16)],
                NI, NI, ES, transpose=True,
            )
            # slabs: f-row r at byte 2*NI*r; f2 = lo ch 212:299 (44 pair-rows)
            F1 = 2 * NI
            F2 = 4 * NI
            # pack leftover (tap, lo-pair) rows: A=[t0 q0:44|t1 q0:44|t2 q0:40],
            # B=[t2 q40:44|t3 q0:44|t4 q0:44]
            pka = pkpool.tile([128, 2 * NI + ETPAD], F8, tag="pka")
            pkb = pkpool.tile([92, 2 * NI + ETPAD], F8, tag="pkb")
            for ci, (dst, r0, q0, q1, k) in enumerate((
                (pka, 0, 0, 44, 0),
                (pka, 44, 0, 44, 1),
                (pka, 88, 0, 40, 2),
                (pkb, 0, 40, 44, 2),
                (pkb, 4, 0, 44, 3),
                (pkb, 48, 0, 44, 4),
            )):
                nq = q1 - q0
                dma_engines[ci % 2].dma_start(
                    out=dst[r0 : r0 + nq, 0 : 2 * NI],
                    in_=et[q0:q1, F2 + 2 * k : F2 + 2 * k + 2 * NI],
                )

            for beta in range(NBLK_G):
                b = NBLK_G * g + beta
                base = 2 * BLK_TOK * beta
                for oi, (o0, o1) in enumerate(CH):
                    ps = pspool.tile([128, 512], F32, tag="ps")
                    s = 0
                    for k in range(K):
                        nc.tensor.matmul(
                            out=ps[:, 0:N],
                            lhsT=wf0_sb[:, k, oi],
                            rhs=dr_rhs(et, base + 2 * k),
                            start=(s == 0), stop=False, perf_mode=DR,
                        )
                        s += 1
                    for k in range(K):
                        nc.tensor.matmul(
                            out=ps[:, 0:N],
                            lhsT=wf1_sb[:, k, oi],
                            rhs=dr_rhs(et, F1 + base + 2 * k),
                            start=False, stop=False, perf_mode=DR,
                        )
                        s += 1
                    nc.tensor.matmul(
                        out=ps[:, 0:N], lhsT=wa_sb[:, oi],
                        rhs=dr_rhs(pka, base),
                        start=False, stop=False, perf_mode=DR,
                    )
                    nc.tensor.matmul(
                        out=ps[:, 0:N], lhsT=wb_sb[:, oi],
                        rhs=dr_rhs(pkb, base),
                        start=False, stop=True, perf_mode=DR,
                    )
                    # max over the 124 valid positions of each sentence:
                    # [o, 4, 124] strided view of the 508-long position axis
                    pav = ps[0 : o1 - o0, :]
                    red_in = bass.AP(
                        pav.tensor, pav.offset,
                        [list(pav.ap[0]), [128, SB], [1, TP]],
                    )
                    cnn_rows = c_sb[oi][0 : o1 - o0] if oi == 2 else c_sb[oi][:]
                    nred += 1
                    nc.vector.tensor_reduce(
                        out=cnn_rows[:, b * SB : (b + 1) * SB],
                        in_=red_in,
                        axis=mybir.AxisListType.X,
                        op=mybir.AluOpType.max,
                    )

        # ---- tail MLP (f32r full-rate), biases folded as ones-rows ----
        ps_h = pspool.tile([NS, D], F32, tag="ps")
        for c in range(len(W2CH)):
            nc.tensor.matmul(
                out=ps_h[:], lhsT=c_sb[c][:], rhs=w2cat_sb[c][:],
                start=(c == 0), stop=(c == len(W2CH) - 1),
            )
        h_sb = const.tile([NS, D], F32R)
        nc.scalar.activation(
            out=h_sb[:], in_=ps_h[:], func=mybir.ActivationFunctionType.Tanh
        )
        ht_sb = []
        for jc, (j0, j1) in enumerate(JCH):
            ps_ht = pspool.tile([100, NS], F32R, tag="ps")
            nc.tensor.transpose(out=ps_ht[:], in_=h_sb[:, j0:j1], identity=ident[:])
            ht = const.tile([100, NS], F32R, tag=f"ht_{j0}", name=f"ht_{j0}")
            nc.scalar.copy(out=ht[:], in_=ps_ht[:])
            ht_sb.append(ht)
        ps_o = pspool.tile([NS, D], F32, tag="ps")
        for jc in range(3):
            nc.tensor.matmul(
                out=ps_o[:], lhsT=ht_sb[jc][:], rhs=w3cat_sb[jc][:],
                start=(jc == 0), stop=False,
            )
        nc.tensor.matmul(
            out=ps_o[:], lhsT=ones_sb[:], rhs=b3row_sb[:], start=False, stop=True
        )
        out_sb = const.tile([NS, D], F32)
        nc.scalar.copy(out=out_sb[:], in_=ps_o[:])
        nc.sync.dma_start(out=out_d[:], in_=out_sb[:])

    nc.finalize()
    return nc


def get_program() -> bass.Bass:
    if "p" not in _PROGRAM_CACHE:
        _PROGRAM_CACHE["p"] = _build_program()
    return _PROGRAM_CACHE["p"]


def _fp8_bytes(x) -> np.ndarray:
    return np.ascontiguousarray(x.astype(E4)).view(np.uint8)


def _prepare_in_maps(inputs: dict) -> list[dict]:
    token_ids = np.asarray(inputs["token_ids"]).astype(np.int64)      # [1024, 128]
    mention = np.asarray(inputs["mention_rep"], dtype=np.float32).reshape(NSENT, D)
    emb = np.asarray(inputs["emb"], dtype=np.float32)
    W1 = np.asarray(inputs["W1"], dtype=np.float64)
    b1 = np.asarray(inputs["b1"], dtype=np.float64)
    conv_w = np.asarray(inputs["conv_w"], dtype=np.float64)           # [o, i, k]
    conv_b = np.asarray(inputs["conv_b"], dtype=np.float64)
    W2 = np.asarray(inputs["W2"], dtype=np.float64)                   # [2D, D]
    b2 = np.asarray(inputs["b2"], dtype=np.float64)
    W3 = np.asarray(inputs["W3"], dtype=np.float32)
    b3 = np.asarray(inputs["b3"], dtype=np.float32)

    Wk = conv_w.transpose(1, 0, 2)                                    # [i, o, k]
    weff = np.stack([W1 @ Wk[:, :, k] for k in range(K)])             # [k, i, o]
    beff = b1 @ Wk.sum(axis=2) + conv_b
    b2eff = b2 + beff @ W2[:D]
    # cnn rows carry 32*cnn on device -> fold 1/32 into W2's cnn rows
    w2_h = W2.copy()
    w2_h[:D] /= 32.0
    w2cat_h = np.concatenate([w2_h, b2eff[None, :]], axis=0).astype(np.float32)
    w3cat_h = np.concatenate(
        [W3.astype(np.float64), b3.astype(np.float64)[None, :]], axis=0
    ).astype(np.float32)

    W32 = (32.0 * weff).astype(E4).astype(np.float32)  # values as quantized
    W1x = weff.astype(E4).astype(np.float32)
    w32b = _fp8_bytes(32.0 * weff).reshape(K, D, D)    # [k, c, o] fp8 bytes
    w1xb = _fp8_bytes(weff).reshape(K, D, D)
    del W32, W1x

    # DoubleRow weight tiles, pre-chunked by o_chunk (contiguous lhsT slices)
    # f0 stream k: partition p, parity j -> hi channel 2p+j, weight fp8(32w)
    wf0_full = np.zeros((128, K, 2, D), np.uint8)
    for j in range(2):
        wf0_full[:, :, j, :] = w32b[:, j::2, :][:, :128, :].transpose(1, 0, 2)
    # f1 stream k: p<22 -> hi ch 256+2p+j; p>=22 -> lo ch 2(p-22)+j, fp8(w)
    wf1_full = np.zeros((128, K, 2, D), np.uint8)
    for j in range(2):
        hi = w32b[:, 256 + j :: 2, :]                  # [k, 22, o]
        wf1_full[:22, :, j, :] = hi.transpose(1, 0, 2)
        lo = w1xb[:, j : 212 : 2, :]                   # [k, 106, o]
        wf1_full[22:128, :, j, :] = lo.transpose(1, 0, 2)
    # packed leftovers: lo ch 212:299 (44 pairs) x 5 taps
    wa_full = np.zeros((128, 2, D), np.uint8)
    wb_full = np.zeros((92, 2, D), np.uint8)
    for j in range(2):
        lo = w1xb[:, 212 + j :: 2, :]                  # [k, 44, o]
        wa_full[0:44, j, :] = lo[0]
        wa_full[44:88, j, :] = lo[1]
        wa_full[88:128, j, :] = lo[2][:40]
        wb_full[0:4, j, :] = lo[2][40:44]
        wb_full[4:48, j, :] = lo[3]
        wb_full[48:92, j, :] = lo[4]

    def chunk_o(w_full):
        # [..., 2, D] -> [..., 3, 2, 128] zero-padded per o_chunk
        pre = w_full.shape[:-2]
        out = np.zeros(pre + (3, 2, 128), np.uint8)
        for oi, (o0, o1) in enumerate(CH):
            out[..., oi, :, 0 : o1 - o0] = w_full[..., :, o0:o1]
        return out

    wf0_h = chunk_o(wf0_full)
    wf1_h = chunk_o(wf1_full)
    wa_h = chunk_o(wa_full)
    wb_h = chunk_o(wb_full)

    idn_h = np.eye(128, dtype=np.float32)

    emb_hi = emb.astype(E4)
    emb_lo = ((emb - emb_hi.astype(np.float32)) * 32.0).astype(E4)
    hi_b = emb_hi.view(np.uint8)                       # [VOCAB, 300]
    lo_b = emb_lo.view(np.uint8)

    in_maps = []
    for c in range(NCORES):
        sl = slice(c * NS, (c + 1) * NS)
        tid_c = token_ids[sl]                          # [128, 128]
        uniq, inv = np.unique(tid_c.ravel(), return_inverse=True)
        tbl_h = np.zeros((NR, ES), np.uint8)
        nu = len(uniq)
        tbl_h[:nu, 0:300] = hi_b[uniq]
        tbl_h[:nu, 300:600] = lo_b[uniq]
        # idx: chunk g, col s, stripe-partition 16a+p -> token g*NI + s*16 + p
        idx16 = (
            inv.astype(np.int16).reshape(GCH, NI // 16, 16)   # [g, s, p]
            .transpose(2, 0, 1).reshape(16, NTOK // 16)       # [p, g*(NI//16)+s]
        )
        idx_h = np.tile(idx16, (8, 1))                   # replicate 8 Q7 stripes
        mt_h = np.ones((D + 1, NS), np.float32)
        mt_h[:D] = mention[sl].T
        in_maps.append(
            {
                "tbl": tbl_h,
                "idx": idx_h,
                "wf0": wf0_h,
                "wf1": wf1_h,
                "wa": wa_h,
                "wb": wb_h,
                "idn": idn_h,
                "w2cat": w2cat_h,
                "w3cat": w3cat_h,
                "mt": mt_h,
            }
        )
    return in_maps


def run(inputs: dict, trace: bool = False, **kwargs):
    nc = get_program()
    in_maps = _prepare_in_maps(inputs)
    res = bass_utils.run_bass_kernel_spmd(
        nc, in_maps, core_ids=list(range(NCORES)), trace=trace, **kwargs
    )
    out = np.concatenate(
        [np.asarray(r["out"]) for r in res.results], axis=0
    ).astype(np.float32)
    return out, res


def kernel(**inputs) -> np.ndarray:
    out, _ = run(inputs)
    return out


# revision 14
# speedup vs baseline: 2.2101x; 1.0554x over previous
"""Trainium2 Bass kernel for nn_CNNEncoder (gather -> lin1 -> conv1d -> maxpool -> MLP).

Strategy (v2: fp8 DoubleRow + transposing dma_gather)
-----------------------------------------------------
Data-parallel over the 1024 = 64*16 sentences: 128 sentences per NeuronCore.

Host-side algebra: lin1 folds into the conv weights (weff[k] = W1 @ Wk), the
constant bias folds into the MLP bias (as in v1).

Precision: the conv runs on the PE in fp8-e4m3 with perf_mode=DoubleRow
(0.5 cycles/row, two 128-row k-tiles per stream).  A single fp8 pass is just
over the error budget, so each embedding row carries a RESIDUAL: the table
row is [fp8(e) | fp8(32*(e - fp8(e)))] and the conv contracts over both the
hi and lo halves (3000 rows -> 12 DoubleRow streams per o_chunk).  Weights:
hi rows use fp8(32*weff), lo rows fp8(weff); the PSUM then holds 32*y and the
1/32 is folded into W2's cnn rows.  Measured end-to-end rel-err ~ 0.019.

Gather: per-core the <=16384 distinct tokens are host-compacted into a
[16384, 768B] fp8 table (int16-indexable), and ONE transpose-mode dma_gather
per 2048-token chunk lands the data channel-major in SBUF (u16-granularity
transpose; channel pairs ride the fp8 parity axis, which matches DoubleRow's
two k-tiles).  This removes all PE transposes and the per-sentence SWDGE
overhead of v1 (8 x ~1.8us Pool instead of 128 x ~1.04us).

Conv: sentences are processed on a 512-token "long axis" (4 sentences/block,
N=508 positions incl. cross-sentence garbage); the max-reduce reads only the
124 valid positions per sentence via a strided AP.  Leftover channelxtap rows
(lo ch 212:299 x 5 taps) are partition-packed by 6 small SBUF->SBUF DMAs so
every o_chunk needs exactly ceil(3000/256) = 12 streams.
"""

import sys

sys.path.insert(0, "/opt/trn_rl_repo")

from contextlib import ExitStack

import ml_dtypes
import numpy as np

import concourse.bass as bass
import concourse.mybir as mybir
import concourse.tile as tile
from concourse import bacc, bass_utils

F32 = mybir.dt.float32
F32R = mybir.dt.float32r
F8 = mybir.dt.float8e4
I16 = mybir.dt.int16
E4 = ml_dtypes.float8_e4m3fn
DR = mybir.MatmulPerfMode.DoubleRow

VOCAB = 100000
D = 300
K = 5
L = 128            # tokens per sentence
NSENT = 1024
NCORES = 8
NS = NSENT // NCORES     # sentences per core = 128
NTOK = NS * L            # tokens per core = 16384
NR = NTOK                # compact table rows (padded)
ES = 768                 # table row bytes: hi 300 | lo 300 | pad 168
GCH = 32                 # gather chunks (one conv block each; NI>768 crashes Q7)
NI = NTOK // GCH         # idxs per gather = 512
BLK_TOK = 512            # tokens per conv block (4 sentences)
NBLK_G = NI // BLK_TOK   # blocks per chunk = 4
SB = BLK_TOK // L        # sentences per block = 4
TP = L - K + 1           # 124 valid conv positions per sentence
N = BLK_TOK - (K - 1)    # 508 conv positions per block stream
ETPAD = 256              # pad: window-read slack + 256B-aligned chunk regions
CH = [(0, 128), (128, 256), (256, 300)]
W2CH = [(0, 128), (128, 256), (256, 384), (384, 512), (512, 601)]
JCH = [(0, 100), (100, 200), (200, 300)]

_PROGRAM_CACHE = {}


def _build_program() -> bass.Bass:
    nc = bacc.Bacc(None, target_bir_lowering=False, dynamic_dma_scratch_size=32768)

    tbl = nc.dram_tensor("tbl", [NR, ES], F8, kind="ExternalInput")
    idx = nc.dram_tensor("idx", [128, NTOK // 16], I16, kind="ExternalInput")
    wf0 = nc.dram_tensor("wf0", [128, K, 3, 2, 128], F8, kind="ExternalInput")
    wf1 = nc.dram_tensor("wf1", [128, K, 3, 2, 128], F8, kind="ExternalInput")
    wa = nc.dram_tensor("wa", [128, 3, 2, 128], F8, kind="ExternalInput")
    wb = nc.dram_tensor("wb", [92, 3, 2, 128], F8, kind="ExternalInput")
    idn = nc.dram_tensor("idn", [128, 128], F32R, kind="ExternalInput")
    w2cat = nc.dram_tensor("w2cat", [2 * D + 1, D], F32R, kind="ExternalInput")
    w3cat = nc.dram_tensor("w3cat", [D + 1, D], F32R, kind="ExternalInput")
    m_t = nc.dram_tensor("mt", [D + 1, NS], F32R, kind="ExternalInput")
    out_d = nc.dram_tensor("out", [NS, D], F32, kind="ExternalOutput")

    with tile.TileContext(nc) as tc, ExitStack() as ctx:
        const = ctx.enter_context(tc.tile_pool(name="const", bufs=1))
        etpool = ctx.enter_context(tc.tile_pool(name="et", bufs=3))
        pkpool = ctx.enter_context(tc.tile_pool(name="pk", bufs=3))
        pspool = ctx.enter_context(tc.tile_pool(name="ps", bufs=8, space="PSUM"))

        idx_sb = const.tile([128, NTOK // 16], I16)
        nc.sync.dma_start(out=idx_sb[:], in_=idx[:])
        wf0_sb = const.tile([128, K, 3, 2, 128], F8)
        nc.sync.dma_start(out=wf0_sb[:], in_=wf0[:])
        wf1_sb = const.tile([128, K, 3, 2, 128], F8)
        nc.sync.dma_start(out=wf1_sb[:], in_=wf1[:])
        wa_sb = const.tile([128, 3, 2, 128], F8)
        nc.sync.dma_start(out=wa_sb[:], in_=wa[:])
        wb_sb = const.tile([92, 3, 2, 128], F8)
        nc.sync.dma_start(out=wb_sb[:], in_=wb[:])
        ident = const.tile([128, 128], F32R)
        nc.sync.dma_start(out=ident[:], in_=idn[:])

        # concat_T tiles [row-chunk, sent] for the tail contraction over the
        # 601-row [32*cnn(300); mention(300); ones] stack.
        c_sb = [
            const.tile([c1 - c0, NS], F32R, tag=f"c_{c0}", name=f"c_{c0}")
            for c0, c1 in W2CH
        ]
        nc.sync.dma_start(out=c_sb[2][44:128, :], in_=m_t[0:84, :])
        nc.sync.dma_start(out=c_sb[3][:], in_=m_t[84:212, :])
        nc.sync.dma_start(out=c_sb[4][:], in_=m_t[212:301, :])

        w2cat_sb = []
        for c0, c1 in W2CH:
            t = const.tile([c1 - c0, D], F32R, tag=f"w2c_{c0}", name=f"w2c_{c0}")
            nc.sync.dma_start(out=t[:], in_=w2cat[c0:c1, :])
            w2cat_sb.append(t)
        w3cat_sb = []
        for j0, j1 in JCH:
            t = const.tile([j1 - j0, D], F32R, tag=f"w3c_{j0}", name=f"w3c_{j0}")
            nc.sync.dma_start(out=t[:], in_=w3cat[j0:j1, :])
            w3cat_sb.append(t)
        b3row_sb = const.tile([1, D], F32R)
        nc.sync.dma_start(out=b3row_sb[:], in_=w3cat[D : D + 1, :])
        ones_sb = const.tile([1, NS], F32R)
        nc.sync.dma_start(out=ones_sb[:], in_=m_t[D : D + 1, :])

        def dr_rhs(tile_ap, base):
            # [128, 2, N] window: parity stride 1, token stride 2
            win = tile_ap[:, base : base + 2 * N]
            return win.rearrange("p (n two) -> p two n", two=2)

        dma_engines = [nc.sync, nc.scalar]
        nred = 0
        # per-chunk region inside the 4-chunk mega tile: 3 f-rows + pad
        CSZ = 3 * 2 * NI + ETPAD          # 3088
        GRP = 4                           # gather chunks per mega tile
        F1 = 2 * NI
        F2 = 4 * NI
        for grp in range(GCH // GRP):
            et = etpool.tile([128, GRP * CSZ], F8, tag="et")
            eap = et[:]
            nc.vector.memset(
                bass.AP(
                    eap.tensor, eap.offset + 3 * 2 * NI,
                    [list(eap.ap[0]), [CSZ, GRP], [1, ETPAD]],
                ),
                0,
            )
            for g4 in range(GRP):
                g = GRP * grp + g4
                gout = et[
                    :, g4 * CSZ : g4 * CSZ + 3 * 2 * NI
                ].rearrange("p (j i) -> p j i", j=6)
                nc.gpsimd.dma_gather(
                    gout, tbl[:], idx_sb[:, g * (NI // 16) : (g + 1) * (NI // 16)],
                    NI, NI, ES, transpose=True,
                )
            # pack leftover (tap, lo-pair) rows for all 4 chunks at once:
            # A=[t0 q0:44|t1 q0:44|t2 q0:40], B=[t2 q40:44|t3 q0:44|t4 q0:44]
            PSZ = 2 * NI + ETPAD
            pka = pkpool.tile([128, GRP, PSZ], F8, tag="pka")
            pkb = pkpool.tile([92, GRP, PSZ], F8, tag="pkb")
            for ci, (dst, r0, q0, q1, k) in enumerate((
                (pka, 0, 0, 44, 0),
                (pka, 44, 0, 44, 1),
                (pka, 88, 0, 40, 2),
                (pkb, 0, 40, 44, 2),
                (pkb, 4, 0, 44, 3),
                (pkb, 48, 0, 44, 4),
            )):
                nq = q1 - q0
                sap = et[q0:q1, :]
                src = bass.AP(
                    sap.tensor, sap.offset + F2 + 2 * k,
                    [list(sap.ap[0]), [CSZ, GRP], [1, 2 * NI]],
                )
                dma_engines[ci % 2].dma_start(
                    out=dst[r0 : r0 + nq, :, 0 : 2 * NI], in_=src
                )

            for g4 in range(GRP):
                b = GRP * grp + g4
                base = g4 * CSZ
                for oi, (o0, o1) in enumerate(CH):
                    ps = pspool.tile([128, 512], F32, tag="ps")
                    s = 0
                    for k in range(K):
                        nc.tensor.matmul(
                            out=ps[:, 0:N],
                            lhsT=wf0_sb[:, k, oi],
                            rhs=dr_rhs(et, base + 2 * k),
                            start=(s == 0), stop=False, perf_mode=DR,
                        )
                        s += 1
                    for k in range(K):
                        nc.tensor.matmul(
                            out=ps[:, 0:N],
                            lhsT=wf1_sb[:, k, oi],
                            rhs=dr_rhs(et, F1 + base + 2 * k),
                            start=False, stop=False, perf_mode=DR,
                        )
                        s += 1
                    nc.tensor.matmul(
                        out=ps[:, 0:N], lhsT=wa_sb[:, oi],
                        rhs=dr_rhs(pka[:, g4, :], 0),
                        start=False, stop=False, perf_mode=DR,
                    )
                    nc.tensor.matmul(
                        out=ps[:, 0:N], lhsT=wb_sb[:, oi],
                        rhs=dr_rhs(pkb[:, g4, :], 0),
                        start=False, stop=True, perf_mode=DR,
                    )
                    # max over the 124 valid positions of each sentence:
                    # [o, 4, 124] strided view of the 508-long position axis
                    pav = ps[0 : o1 - o0, :]
                    red_in = bass.AP(
                        pav.tensor, pav.offset,
                        [list(pav.ap[0]), [128, SB], [1, TP]],
                    )
                    cnn_rows = c_sb[oi][0 : o1 - o0] if oi == 2 else c_sb[oi][:]
                    nred += 1
                    nc.vector.tensor_reduce(
                        out=cnn_rows[:, b * SB : (b + 1) * SB],
                        in_=red_in,
                        axis=mybir.AxisListType.X,
                        op=mybir.AluOpType.max,
                    )

        # ---- tail MLP (f32r full-rate), biases folded as ones-rows ----
        ps_h = pspool.tile([NS, D], F32, tag="ps")
        for c in range(len(W2CH)):
            nc.tensor.matmul(
                out=ps_h[:], lhsT=c_sb[c][:], rhs=w2cat_sb[c][:],
                start=(c == 0), stop=(c == len(W2CH) - 1),
            )
        h_sb = const.tile([NS, D], F32R)
        nc.scalar.activation(
            out=h_sb[:], in_=ps_h[:], func=mybir.ActivationFunctionType.Tanh
        )
        ht_sb = []
        for jc, (j0, j1) in enumerate(JCH):
            ps_ht = pspool.tile([100, NS], F32R, tag="ps")
            nc.tensor.transpose(out=ps_ht[:], in_=h_sb[:, j0:j1], identity=ident[:])
            ht = const.tile([100, NS], F32R, tag=f"ht_{j0}", name=f"ht_{j0}")
            nc.scalar.copy(out=ht[:], in_=ps_ht[:])
            ht_sb.append(ht)
        ps_o = pspool.tile([NS, D], F32, tag="ps")
        for jc in range(3):
            nc.tensor.matmul(
                out=ps_o[:], lhsT=ht_sb[jc][:], rhs=w3cat_sb[jc][:],
                start=(jc == 0), stop=False,
            )
        nc.tensor.matmul(
            out=ps_o[:], lhsT=ones_sb[:], rhs=b3row_sb[:], start=False, stop=True
        )
        out_sb = const.tile([NS, D], F32)
        nc.scalar.copy(out=out_sb[:], in_=ps_o[:])
        nc.sync.dma_start(out=out_d[:], in_=out_sb[:])

    nc.finalize()
    return nc


def get_program() -> bass.Bass:
    if "p" not in _PROGRAM_CACHE:
        _PROGRAM_CACHE["p"] = _build_program()
    return _PROGRAM_CACHE["p"]


def _fp8_bytes(x) -> np.ndarray:
    return np.ascontiguousarray(x.astype(E4)).view(np.uint8)


def _prepare_in_maps(inputs: dict) -> list[dict]:
    token_ids = np.asarray(inputs["token_ids"]).astype(np.int64)      # [1024, 128]
    mention = np.asarray(inputs["mention_rep"], dtype=np.float32).reshape(NSENT, D)
    emb = np.asarray(inputs["emb"], dtype=np.float32)
    W1 = np.asarray(inputs["W1"], dtype=np.float64)
    b1 = np.asarray(inputs["b1"], dtype=np.float64)
    conv_w = np.asarray(inputs["conv_w"], dtype=np.float64)           # [o, i, k]
    conv_b = np.asarray(inputs["conv_b"], dtype=np.float64)
    W2 = np.asarray(inputs["W2"], dtype=np.float64)                   # [2D, D]
    b2 = np.asarray(inputs["b2"], dtype=np.float64)
    W3 = np.asarray(inputs["W3"], dtype=np.float32)
    b3 = np.asarray(inputs["b3"], dtype=np.float32)

    Wk = conv_w.transpose(1, 0, 2)                                    # [i, o, k]
    weff = np.stack([W1 @ Wk[:, :, k] for k in range(K)])             # [k, i, o]
    beff = b1 @ Wk.sum(axis=2) + conv_b
    b2eff = b2 + beff @ W2[:D]
    # cnn rows carry 32*cnn on device -> fold 1/32 into W2's cnn rows
    w2_h = W2.copy()
    w2_h[:D] /= 32.0
    w2cat_h = np.concatenate([w2_h, b2eff[None, :]], axis=0).astype(np.float32)
    w3cat_h = np.concatenate(
        [W3.astype(np.float64), b3.astype(np.float64)[None, :]], axis=0
    ).astype(np.float32)

    W32 = (32.0 * weff).astype(E4).astype(np.float32)  # values as quantized
    W1x = weff.astype(E4).astype(np.float32)
    w32b = _fp8_bytes(32.0 * weff).reshape(K, D, D)    # [k, c, o] fp8 bytes
    w1xb = _fp8_bytes(weff).reshape(K, D, D)
    del W32, W1x

    # DoubleRow weight tiles, pre-chunked by o_chunk (contiguous lhsT slices)
    # f0 stream k: partition p, parity j -> hi channel 2p+j, weight fp8(32w)
    wf0_full = np.zeros((128, K, 2, D), np.uint8)
    for j in range(2):
        wf0_full[:, :, j, :] = w32b[:, j::2, :][:, :128, :].transpose(1, 0, 2)
    # f1 stream k: p<22 -> hi ch 256+2p+j; p>=22 -> lo ch 2(p-22)+j, fp8(w)
    wf1_full = np.zeros((128, K, 2, D), np.uint8)
    for j in range(2):
        hi = w32b[:, 256 + j :: 2, :]                  # [k, 22, o]
        wf1_full[:22, :, j, :] = hi.transpose(1, 0, 2)
        lo = w1xb[:, j : 212 : 2, :]                   # [k, 106, o]
        wf1_full[22:128, :, j, :] = lo.transpose(1, 0, 2)
    # packed leftovers: lo ch 212:299 (44 pairs) x 5 taps
    wa_full = np.zeros((128, 2, D), np.uint8)
    wb_full = np.zeros((92, 2, D), np.uint8)
    for j in range(2):
        lo = w1xb[:, 212 + j :: 2, :]                  # [k, 44, o]
        wa_full[0:44, j, :] = lo[0]
        wa_full[44:88, j, :] = lo[1]
        wa_full[88:128, j, :] = lo[2][:40]
        wb_full[0:4, j, :] = lo[2][40:44]
        wb_full[4:48, j, :] = lo[3]
        wb_full[48:92, j, :] = lo[4]

    def chunk_o(w_full):
        # [..., 2, D] -> [..., 3, 2, 128] zero-padded per o_chunk
        pre = w_full.shape[:-2]
        out = np.zeros(pre + (3, 2, 128), np.uint8)
        for oi, (o0, o1) in enumerate(CH):
            out[..., oi, :, 0 : o1 - o0] = w_full[..., :, o0:o1]
        return out

    wf0_h = chunk_o(wf0_full)
    wf1_h = chunk_o(wf1_full)
    wa_h = chunk_o(wa_full)
    wb_h = chunk_o(wb_full)

    idn_h = np.eye(128, dtype=np.float32)

    emb_hi = emb.astype(E4)
    emb_lo = ((emb - emb_hi.astype(np.float32)) * 32.0).astype(E4)
    hi_b = emb_hi.view(np.uint8)                       # [VOCAB, 300]
    lo_b = emb_lo.view(np.uint8)

    in_maps = []
    for c in range(NCORES):
        sl = slice(c * NS, (c + 1) * NS)
        tid_c = token_ids[sl]                          # [128, 128]
        uniq, inv = np.unique(tid_c.ravel(), return_inverse=True)
        tbl_h = np.zeros((NR, ES), np.uint8)
        nu = len(uniq)
        tbl_h[:nu, 0:300] = hi_b[uniq]
        tbl_h[:nu, 300:600] = lo_b[uniq]
        # idx: chunk g, col s, stripe-partition 16a+p -> token g*NI + s*16 + p
        idx16 = (
            inv.astype(np.int16).reshape(GCH, NI // 16, 16)   # [g, s, p]
            .transpose(2, 0, 1).reshape(16, NTOK // 16)       # [p, g*(NI//16)+s]
        )
        idx_h = np.tile(idx16, (8, 1))                   # replicate 8 Q7 stripes
        mt_h = np.ones((D + 1, NS), np.float32)
        mt_h[:D] = mention[sl].T
        in_maps.append(
            {
                "tbl": tbl_h,
                "idx": idx_h,
                "wf0": wf0_h,
                "wf1": wf1_h,
                "wa": wa_h,
                "wb": wb_h,
                "idn": idn_h,
                "w2cat": w2cat_h,
                "w3cat": w3cat_h,
                "mt": mt_h,
            }
        )
    return in_maps


def run(inputs: dict, trace: bool = False, **kwargs):
    nc = get_program()
    in_maps = _prepare_in_maps(inputs)
    res = bass_utils.run_bass_kernel_spmd(
        nc, in_maps, core_ids=list(range(NCORES)), trace=trace, **kwargs
    )
    out = np.concatenate(
        [np.asarray(r["out"]) for r in res.results], axis=0
    ).astype(np.float32)
    return out, res


def kernel(**inputs) -> np.ndarray:
    out, _ = run(inputs)
    return out
